# revision 1
# baseline (speedup 1.0000x reference)
"""Trainium2 kernel for nn_MHAttention_15358803050646.

The reference module computes
    qkv = qkv_w @ x + qkv_b          (1x1 conv over channels)
    q, k, v = split(qkv)
    att = softmax(q @ k^T / sqrt(d_k))
    out = einsum('bnqk,bnqd->bnqd', att, v)      # <-- sums att over k
    out = out_w @ out + out_b

The einsum 'bnqk,bnqd->bnqd' multiplies v elementwise by the softmax
row-sum, which is identically 1.  The whole attention block is therefore
the identity on v, and the network collapses algebraically to

    out = out_w @ (v_w @ x + v_b) + out_b = W_eff @ x + b_eff

with v_w = qkv_w[1024:1536], v_b = qkv_b[1024:1536].  We fuse the two
channel matrices on the host (512x512x512 fp32, sub-millisecond) and run
a single 512x512 channel projection over all pixels on device.

Sharding: data-parallel over batch — B == 8 images, one per NeuronCore.
Per core: out[o, p] = sum_c W_eff[o, c] * x[c, p] + b_eff[o] with
C = 512 channels and HW = 1024 pixels, i.e. a 512x512x1024 matmul.

Matmul precision ("fp16x2" mode, default): the TRN2 PE runs fp32 matmuls
at 4 cycles/row but fp16 at 1 cycle/row.  Each fp32 operand is split on
the host into an fp16 high part and an fp16 residual (hi = fp16(a),
lo = fp16(a - hi)); the product is computed as three fp16 matmuls
Wh@Xh + Wh@Xl + Wl@Xh accumulated in fp32 PSUM.  fp16 has 11 mantissa
bits, so hi+lo carries ~22 bits and the dropped Wl@Xl term is O(2^-24)
relative: measured end-to-end relative error is ~4e-7, the same as a
plain fp32 matmul, at 3/4 the PE cost and identical DMA bytes.

Device layouts are packed on the host so every DMA is 128 partitions x
contiguous bytes per partition.
"""

import numpy as np

import concourse.mybir as mybir
import concourse.tile as tile
from concourse import bacc
from concourse.bass_utils import run_bass_kernel_spmd

P = 128          # SBUF partitions
C = 512          # model channels
HW = 1024        # pixels per image (32*32)
B = 8            # batch == number of cores
KO = C // P      # contraction chunks (4)
MO = C // P      # output-channel chunks (4)
N_TILE = 512     # pixels per PSUM tile (one fp32 PSUM bank)
N_TILES = HW // N_TILE

_FP32 = mybir.dt.float32
_FP16 = mybir.dt.float16


def _build_fp16x2(nc):
    """3-term fp16 split-matmul kernel body (see module docstring).

    Schedule notes (cost-model driven):
    - All DMA transfers serialize on the shared SDMA engines (~360 GB/s), so
      the first matmul can only start once its operands' transfers finish.
      The n=0 operands are therefore loaded in P-sized k-chunks, interleaved
      hi-before-lo in the order the PE consumes them, letting PE start after
      ~256 KB instead of ~2 MB.
    - Input DMAs are issued from several engines (SP: hi stream, ACT: lo
      stream + bias, Pool/SWDGE: trailing lo tile) so per-DMA issue cost does
      not serialize behind one sequencer.
    - Output DMAs are issued from the Activation engine: each directly
      follows its bias-add activation in program order, needing no semaphore.
    - n=0 uses k-outer matmul order (stream-friendly); the last n-tile uses
      m-outer order so the four output groups finish staggered and the tail
      only waits for one small DMA.
    """
    wh = nc.declare_dram_parameter("wh", [P, KO * C], _FP16, isOutput=False)
    wl = nc.declare_dram_parameter("wl", [P, KO * C], _FP16, isOutput=False)
    bias = nc.declare_dram_parameter("bias", [P, MO], _FP32, isOutput=False)
    # x*[n*P + p, ko*N_TILE + j] = x_core[ko*P + p, n*N_TILE + j]
    xh = nc.declare_dram_parameter("xh", [N_TILES * P, KO * N_TILE], _FP16, isOutput=False)
    xl = nc.declare_dram_parameter("xl", [N_TILES * P, KO * N_TILE], _FP16, isOutput=False)
    # out[(n*MO + m)*P + p, j] = out_core[m*P + p, n*N_TILE + j]
    out = nc.declare_dram_parameter("out", [N_TILES * MO * P, N_TILE], _FP32, isOutput=True)

    wh_r = wh.rearrange("p (ko o) -> p ko o", ko=KO)
    wl_r = wl.rearrange("p (ko o) -> p ko o", ko=KO)

    with tile.TileContext(nc) as tc:
        with (
            tc.tile_pool(name="wpool", bufs=1) as wpool,
            tc.tile_pool(name="xpool", bufs=2) as xpool,
            tc.tile_pool(name="opool", bufs=4) as opool,
            tc.tile_pool(name="psum", bufs=8, space="PSUM") as psum_pool,
        ):
            b_sb = wpool.tile([P, MO], _FP32, tag="bias")
            nc.scalar.dma_start(b_sb[:], bias[:])

            # n=0 operands, k-chunked, in PE consumption order.
            wh_k = [wpool.tile([P, C], _FP16, tag=f"wh{k}", name=f"wh_k{k}") for k in range(KO)]
            wl_k = [wpool.tile([P, C], _FP16, tag=f"wl{k}", name=f"wl_k{k}") for k in range(KO)]
            xh0_k = [xpool.tile([P, N_TILE], _FP16, tag=f"xh0_{k}", name=f"xh0_k{k}") for k in range(KO)]
            xl0_k = [xpool.tile([P, N_TILE], _FP16, tag=f"xl0_{k}", name=f"xl0_k{k}") for k in range(KO)]
            for k in range(KO):
                nc.sync.dma_start(wh_k[k][:], wh_r[:, k])
                nc.sync.dma_start(xh0_k[k][:], xh[0:P, k * N_TILE:(k + 1) * N_TILE])
            for k in range(KO):
                nc.scalar.dma_start(wl_k[k][:], wl_r[:, k])
                nc.scalar.dma_start(xl0_k[k][:], xl[0:P, k * N_TILE:(k + 1) * N_TILE])

            # Remaining n-tiles: whole-tile loads (they arrive long before use).
            x_rest = []
            for n in range(1, N_TILES):
                xh_sb = xpool.tile([P, KO, N_TILE], _FP16, tag="xh")
                nc.sync.dma_start(
                    xh_sb[:], xh[n * P:(n + 1) * P].rearrange("p (ko j) -> p ko j", ko=KO))
                xl_sb = xpool.tile([P, KO, N_TILE], _FP16, tag="xl")
                nc.gpsimd.dma_start(
                    xl_sb[:], xl[n * P:(n + 1) * P].rearrange("p (ko j) -> p ko j", ko=KO))
                x_rest.append((xh_sb, xl_sb))

            def emit_group_tail(n, m, ps):
                o_sb = opool.tile([P, N_TILE], _FP32, tag="o")
                row = (n * MO + m) * P
                # out = psum + bias[o], PSUM -> SBUF on the scalar engine
                nc.scalar.activation(
                    o_sb[:], ps[:], mybir.ActivationFunctionType.Identity,
                    bias=b_sb[:, m:m + 1])
                nc.scalar.dma_start(out[row:row + P], o_sb[:])

            # n=0: k-outer, hi terms first, 4 psum groups in flight.
            ps0 = [psum_pool.tile([P, N_TILE], _FP32, tag="ps", name=f"ps0_{m}") for m in range(MO)]
            for k in range(KO):
                for m in range(MO):
                    nc.tensor.matmul(ps0[m][:], lhsT=wh_k[k][:, m * P:(m + 1) * P],
                                     rhs=xh0_k[k][:], start=(k == 0), stop=False)
            for k in range(KO):
                for m in range(MO):
                    nc.tensor.matmul(ps0[m][:], lhsT=wl_k[k][:, m * P:(m + 1) * P],
                                     rhs=xh0_k[k][:], start=False, stop=False)
            for k in range(KO):
                for m in range(MO):
                    nc.tensor.matmul(ps0[m][:], lhsT=wh_k[k][:, m * P:(m + 1) * P],
                                     rhs=xl0_k[k][:], start=False, stop=(k == KO - 1))
            for m in range(MO):
                emit_group_tail(0, m, ps0[m])

            # n>=1: m-outer so groups retire staggered.  The very last
            # m-group runs as two half-width (N/2) psum groups: the final
            # ACT -> out-DMA chain is then half-length and starts 12
            # half-matmuls earlier, trimming the kernel tail.
            for n in range(1, N_TILES):
                xh_sb, xl_sb = x_rest[n - 1]
                for m in range(MO):
                    om = slice(m * P, (m + 1) * P)
                    last_group = n == N_TILES - 1 and m == MO - 1
                    halves = (
                        [(slice(0, N_TILE // 2), 0), (slice(N_TILE // 2, N_TILE), 1)]
                        if last_group else [(slice(0, N_TILE), None)]
                    )
                    for js, half in halves:
                        ps = psum_pool.tile([P, js.stop - js.start], _FP32, tag="ps",
                                            name=f"ps_{n}_{m}_{half}")
                        for k in range(KO):
                            nc.tensor.matmul(ps[:], lhsT=wh_k[k][:, om],
                                             rhs=xh_sb[:, k, js],
                                             start=(k == 0), stop=False)
                        for k in range(KO):
                            nc.tensor.matmul(ps[:], lhsT=wl_k[k][:, om],
                                             rhs=xh_sb[:, k, js],
                                             start=False, stop=False)
                        for k in range(KO):
                            nc.tensor.matmul(ps[:], lhsT=wh_k[k][:, om],
                                             rhs=xl_sb[:, k, js],
                                             start=False, stop=(k == KO - 1))
                        o_sb = opool.tile([P, js.stop - js.start], _FP32, tag="o",
                                          name=f"o_{n}_{m}_{half}")
                        nc.scalar.activation(
                            o_sb[:], ps[:], mybir.ActivationFunctionType.Identity,
                            bias=b_sb[:, m:m + 1])
                        row = (n * MO + m) * P
                        if half == 0:
                            # keep ACT's sequencer free for the final
                            # activation: the first half's store goes via SP
                            nc.sync.dma_start(out[row:row + P, js], o_sb[:])
                        else:
                            nc.scalar.dma_start(out[row:row + P, js], o_sb[:])


def _build_fp32(nc, mm_dtype):
    """Single-dtype kernel body (fp32 or f32r matmuls)."""
    w = nc.declare_dram_parameter("w", [P, KO * C], mm_dtype, isOutput=False)
    bias = nc.declare_dram_parameter("bias", [P, MO], _FP32, isOutput=False)
    x = nc.declare_dram_parameter("x", [N_TILES * P, KO * N_TILE], mm_dtype, isOutput=False)
    out = nc.declare_dram_parameter("out", [N_TILES * MO * P, N_TILE], _FP32, isOutput=True)

    with tile.TileContext(nc) as tc:
        with (
            tc.tile_pool(name="wpool", bufs=1) as wpool,
            tc.tile_pool(name="xpool", bufs=N_TILES) as xpool,
            tc.tile_pool(name="opool", bufs=4) as opool,
            tc.tile_pool(name="psum", bufs=8, space="PSUM") as psum_pool,
        ):
            w_sb = wpool.tile([P, KO, C], mm_dtype, tag="w")
            nc.sync.dma_start(w_sb[:], w.rearrange("p (ko o) -> p ko o", ko=KO))
            x_sbs = []
            for n in range(N_TILES):
                x_sb = xpool.tile([P, KO, N_TILE], mm_dtype, tag="x")
                nc.sync.dma_start(
                    x_sb[:], x[n * P:(n + 1) * P].rearrange("p (ko j) -> p ko j", ko=KO))
                x_sbs.append(x_sb)
                if n == 0:
                    b_sb = wpool.tile([P, MO], _FP32, tag="bias")
                    nc.sync.dma_start(b_sb[:], bias[:])

            for n in range(N_TILES):
                x_sb = x_sbs[n]
                for m in range(MO):
                    ps = psum_pool.tile([P, N_TILE], _FP32, tag="ps")
                    for k in range(KO):
                        nc.tensor.matmul(
                            ps[:], lhsT=w_sb[:, k, m * P:(m + 1) * P], rhs=x_sb[:, k, :],
                            start=(k == 0), stop=(k == KO - 1))
                    o_sb = opool.tile([P, N_TILE], _FP32, tag="o")
                    nc.scalar.activation(
                        o_sb[:], ps[:], mybir.ActivationFunctionType.Identity,
                        bias=b_sb[:, m:m + 1])
                    nc.sync.dma_start(out[(n * MO + m) * P:(n * MO + m + 1) * P], o_sb[:])


def _build_bass(mode="fp16x2"):
    # Bacc (not plain Bass): its finalize() runs the legalization passes that
    # split multi-semaphore waits (TRN2 allows one sync wait per instruction).
    nc = bacc.Bacc()
    if mode == "fp16x2":
        _build_fp16x2(nc)
    elif mode == "fp32":
        _build_fp32(nc, _FP32)
    elif mode == "f32r":
        _build_fp32(nc, mybir.dt.float32r)
    else:
        raise ValueError(mode)
    # Runs Bacc.compile(): moves matmul waits to ldweights, splits multi-sem
    # waits into event semaphores, allocates registers.
    nc.finalize()
    return nc


def _pack_w(w2d):
    # [C, C] (transposed W_eff: w2d[c, o]) -> [P, KO*C] with [p, ko, o] layout
    return np.ascontiguousarray(
        w2d.reshape(KO, P, C).transpose(1, 0, 2)).reshape(P, KO * C)


def _pack_x(xm):
    # [B, C, HW] -> [B, N_TILES*P, KO*N_TILE] with [n, p, ko, j] layout
    t = xm.reshape(B, KO, P, N_TILES, N_TILE).transpose(0, 3, 2, 1, 4)
    return np.ascontiguousarray(t).reshape(B, N_TILES * P, KO * N_TILE)


_NC_CACHE = {}


def _get_nc(mode):
    if mode not in _NC_CACHE:
        _NC_CACHE[mode] = _build_bass(mode)
    return _NC_CACHE[mode]


MODE = "fp16x2"


def kernel(x, qkv_w, qkv_b, out_w, out_b):
    x = np.asarray(x, dtype=np.float32)
    qkv_w = np.asarray(qkv_w, dtype=np.float32)
    qkv_b = np.asarray(qkv_b, dtype=np.float32)
    out_w = np.asarray(out_w, dtype=np.float32)
    out_b = np.asarray(out_b, dtype=np.float32)

    Bx, Cx, Hx, Wx = x.shape
    assert (Bx, Cx, Hx * Wx) == (B, C, HW), (x.shape,)

    # Host-side algebraic fusion (see module docstring).
    v_w = qkv_w[2 * C:3 * C]
    v_b = qkv_b[2 * C:3 * C]
    w_eff = out_w @ v_w                    # [C, C]
    b_eff = out_w @ v_b + out_b            # [C]

    bias_host = np.ascontiguousarray(b_eff.reshape(MO, P).T)
    xm = x.reshape(B, C, HW)
    wt = np.ascontiguousarray(w_eff.T)     # wt[c, o]

    nc = _get_nc(MODE)
    if MODE == "fp16x2":
        wt_h = wt.astype(np.float16)
        wt_l = (wt - wt_h.astype(np.float32)).astype(np.float16)
        x_h16 = xm.astype(np.float16)
        x_l16 = (xm - x_h16.astype(np.float32)).astype(np.float16)
        wh_host = _pack_w(wt_h)
        wl_host = _pack_w(wt_l)
        xh_host = _pack_x(x_h16)
        xl_host = _pack_x(x_l16)
        in_maps = [
            {"wh": wh_host, "wl": wl_host, "bias": bias_host,
             "xh": xh_host[i], "xl": xl_host[i]}
            for i in range(B)
        ]
    else:
        w_host = _pack_w(wt)
        x_dev = _pack_x(xm)
        in_maps = [{"w": w_host, "bias": bias_host, "x": x_dev[i]} for i in range(B)]

    res = run_bass_kernel_spmd(nc, in_maps, core_ids=list(range(B)))

    # out rows [(n*MO + m)*P + p] hold out_core[m*P + p, n*N_TILE:(n+1)*N_TILE]
    out_dev = np.stack([res.results[i]["out"] for i in range(B)], axis=0)
    out_dev = out_dev.reshape(B, N_TILES, MO, P, N_TILE)
    out_full = out_dev.transpose(0, 2, 3, 1, 4).reshape(B, C, Hx, Wx)
    return np.ascontiguousarray(out_full.astype(np.float32))



# revision 3
# speedup vs baseline: 1.9336x; 1.9336x over previous
"""Trainium2 kernel for nn_MHAttention_15358803050646.

The reference module computes
    qkv = qkv_w @ x + qkv_b          (1x1 conv over channels)
    q, k, v = split(qkv)
    att = softmax(q @ k^T / sqrt(d_k))
    out = einsum('bnqk,bnqd->bnqd', att, v)      # <-- sums att over k
    out = out_w @ out + out_b

The einsum 'bnqk,bnqd->bnqd' multiplies v elementwise by the softmax
row-sum, which is identically 1.  The whole attention block is therefore
the identity on v, and the network collapses algebraically to

    out = out_w @ (v_w @ x + v_b) + out_b = W_eff @ x + b_eff

with v_w = qkv_w[1024:1536], v_b = qkv_b[1024:1536].  We fuse the two
channel matrices on the host (512x512x512 fp32, sub-millisecond) and run
a single 512x512 channel projection over all pixels on device.

Sharding: data-parallel over batch — B == 8 images, one per NeuronCore.
Per core: out[o, p] = sum_c W_eff[o, c] * x[c, p] + b_eff[o] with
C = 512 channels and HW = 1024 pixels, i.e. a 512x512x1024 matmul.

Precision: the harness tolerance is rel_err < 2e-2; a single fp16
matmul term (fp32 PSUM accumulation) gives ~5e-4, so W and x ship as
fp16 and the PE does one pass (16384 rows) instead of the fp32-accurate
3-term split (49152 rows).  Outputs return as fp16 (exactly upcast on
the host), halving writeback DMA.

Schedule (cost-model driven):
- Phase A = pixels 0..511, k-outer: 4 PSUM groups accumulate across the
  4 k-chunks as they stream in; all 4 finish at the end of k3 and
  evacuate mid-kernel.
- Phase B = pixels 512..1023 as 4 slices of 128 px, m-major per slice:
  groups retire staggered so outputs drain while the PE still computes;
  the last slice gives a short kernel tail.
- All input DMAs issue from SP in consumption order (HWDGE serializes
  descriptor generation at ~500ns per DMA, so issue order == bus order).
- PSUM evacuation (bias add + fp32->fp16 cast) alternates between the
  Activation and Vector engines so neither becomes the tail.
"""

import numpy as np

import concourse.mybir as mybir
import concourse.tile as tile
from concourse import bacc
from concourse.bass_utils import run_bass_kernel_spmd

P = 128          # SBUF partitions
C = 512          # model channels
HW = 1024        # pixels per image (32*32)
B = 8            # batch == number of cores
KO = C // P      # contraction chunks (4)
MO = C // P      # output-channel chunks (4)
PXA = 512        # phase A pixels (one PSUM bank wide)
NSB = 4          # phase B slices
PXS = (HW - PXA) // NSB   # pixels per phase B slice (128)

_FP32 = mybir.dt.float32
_FP16 = mybir.dt.float16

_ID = mybir.ActivationFunctionType.Identity


def _build_fp16(nc):
    w = nc.declare_dram_parameter("w", [P, KO * C], _FP16, isOutput=False)
    bias = nc.declare_dram_parameter("bias", [P, MO], _FP32, isOutput=False)
    # xa[k*P + p, j]      = x_core[k*128+p, j]            j in [0, 512)
    xa = nc.declare_dram_parameter("xa", [KO * P, PXA], _FP16, isOutput=False)
    # xb[s*P + p, k*PXS+j] = x_core[k*128+p, 512+s*128+j]
    xb = nc.declare_dram_parameter("xb", [NSB * P, KO * PXS], _FP16, isOutput=False)
    # outa[m*P + p, j]       = y[m*128+p, j]              j in [0, 512)
    outa = nc.declare_dram_parameter("outa", [MO * P, PXA], _FP16, isOutput=True)
    # outb[s*P + p, m*PXS+j] = y[m*128+p, 512+s*128+j]
    outb = nc.declare_dram_parameter("outb", [NSB * P, MO * PXS], _FP16, isOutput=True)

    with tile.TileContext(nc) as tc:
        with (
            tc.tile_pool(name="wpool", bufs=1) as wpool,
            tc.tile_pool(name="xpool", bufs=1) as xpool,
            tc.tile_pool(name="opool", bufs=1) as opool,
            tc.tile_pool(name="psa", bufs=4, space="PSUM") as psa_pool,
            tc.tile_pool(name="psb", bufs=4, space="PSUM") as psb_pool,
        ):
            w_sb = wpool.tile([P, KO, C], _FP16, tag="w")
            b_sb = wpool.tile([P, MO], _FP32, tag="bias")
            xa_sb = [xpool.tile([P, PXA], _FP16, tag=f"xa{k}", name=f"xa{k}")
                     for k in range(KO)]
            xb_sb = [xpool.tile([P, 2, KO * PXS], _FP16, tag=f"xb{g}", name=f"xb{g}")
                     for g in range(2)]
            oa_sb = [opool.tile([P, 2, PXA], _FP16, tag=f"oa{h}", name=f"oa{h}")
                     for h in range(2)]
            ob_sb = [opool.tile([P, MO * PXS], _FP16, tag=f"ob{s}", name=f"ob{s}")
                     for s in range(NSB)]

            # --- input DMA stream (SP queue; issue order == bus order) ----
            w_r = w.rearrange("p (ko o) -> p ko o", ko=KO)

            def w_dma(k):
                nc.sync.dma_start(w_sb[:, k, :], w_r[:, k])

            def xa_dma(k):
                nc.sync.dma_start(xa_sb[k][:], xa[k * P:(k + 1) * P])

            def xb_dma(g):
                src = xb[2 * g * P:(2 * g + 2) * P]
                nc.sync.dma_start(
                    xb_sb[g][:], src.rearrange("(s p) c -> p s c", s=2))

            w_dma(0)
            xa_dma(0)
            w_dma(1)
            xa_dma(1)
            xa_dma(2)
            w_dma(2)
            w_dma(3)
            xa_dma(3)
            xb_dma(0)
            xb_dma(1)
            # bias: tiny, from ACT queue (after its act-table load)
            nc.scalar.dma_start(b_sb[:], bias[:])

            # --- phase A: k-outer, 4 wide PSUM groups -------------------
            ps_a = [psa_pool.tile([P, PXA], _FP32, tag="psa", name=f"psa{m}")
                    for m in range(MO)]
            for k in range(KO):
                for m in range(MO):
                    nc.tensor.matmul(
                        ps_a[m][:], lhsT=w_sb[:, k, m * P:(m + 1) * P],
                        rhs=xa_sb[k][:], start=(k == 0), stop=(k == KO - 1))

            # evacuate: bias add + cast, ACT/DVE alternating
            for m in range(MO):
                dst = oa_sb[m // 2][:, m % 2, :]
                if m % 2 == 0:
                    nc.scalar.activation(dst, ps_a[m][:], _ID,
                                         bias=b_sb[:, m:m + 1])
                else:
                    nc.vector.tensor_scalar_add(dst, ps_a[m][:],
                                                b_sb[:, m:m + 1])
            # outa in two halves (engine: SP is idle again by now)
            for h in range(2):
                nc.sync.dma_start(
                    outa[2 * h * P:(2 * h + 2) * P].rearrange(
                        "(m p) j -> p m j", m=2),
                    oa_sb[h][:])

            # --- phase B: 4 slices of 128 px, m-major -------------------
            out_eng = [nc.scalar, nc.sync, nc.scalar, nc.sync]
            for s in range(NSB):
                ps = psb_pool.tile([P, MO, PXS], _FP32, tag="psb", name=f"psb{s}")
                g, sl = divmod(s, 2)
                for m in range(MO):
                    for k in range(KO):
                        nc.tensor.matmul(
                            ps[:, m, :], lhsT=w_sb[:, k, m * P:(m + 1) * P],
                            rhs=xb_sb[g][:, sl, k * PXS:(k + 1) * PXS],
                            start=(k == 0), stop=(k == KO - 1))
                    dst = ob_sb[s][:, m * PXS:(m + 1) * PXS]
                    if (s + m) % 2 == 0:
                        nc.scalar.activation(dst, ps[:, m, :], _ID,
                                             bias=b_sb[:, m:m + 1])
                    else:
                        nc.vector.tensor_scalar_add(dst, ps[:, m, :],
                                                    b_sb[:, m:m + 1])
                out_eng[s].dma_start(outb[s * P:(s + 1) * P], ob_sb[s][:])


def _build_bass(mode="fp16"):
    # Bacc (not plain Bass): its finalize() runs the legalization passes that
    # split multi-semaphore waits (TRN2 allows one sync wait per instruction).
    nc = bacc.Bacc()
    _build_fp16(nc)
    nc.finalize()
    return nc


_NC_CACHE = {}


def _get_nc(mode):
    if mode not in _NC_CACHE:
        _NC_CACHE[mode] = _build_bass(mode)
    return _NC_CACHE[mode]


MODE = "fp16"


def _pack_w(w2d):
    # [C, C] (transposed W_eff: w2d[c, o]) -> [P, KO*C] with [p, ko, o] layout
    return np.ascontiguousarray(
        w2d.reshape(KO, P, C).transpose(1, 0, 2)).reshape(P, KO * C)


def kernel(x, qkv_w, qkv_b, out_w, out_b):
    x = np.asarray(x, dtype=np.float32)
    qkv_w = np.asarray(qkv_w, dtype=np.float32)
    qkv_b = np.asarray(qkv_b, dtype=np.float32)
    out_w = np.asarray(out_w, dtype=np.float32)
    out_b = np.asarray(out_b, dtype=np.float32)

    Bx, Cx, Hx, Wx = x.shape
    assert (Bx, Cx, Hx * Wx) == (B, C, HW), (x.shape,)

    # Host-side algebraic fusion (see module docstring).
    v_w = qkv_w[2 * C:3 * C]
    v_b = qkv_b[2 * C:3 * C]
    w_eff = out_w @ v_w                    # [C, C]
    b_eff = out_w @ v_b + out_b            # [C]

    bias_host = np.ascontiguousarray(b_eff.reshape(MO, P).T)
    xm = x.reshape(B, C, HW).astype(np.float16)
    w_host = _pack_w(np.ascontiguousarray(w_eff.T).astype(np.float16))

    # xa[b, k*P+p, j] = xm[b, k*128+p, j], j in [0,512)
    xa_host = np.ascontiguousarray(xm[:, :, :PXA]).reshape(B, KO * P, PXA)
    # xb[b, s*P+p, k*PXS+j] = xm[b, k*128+p, PXA + s*PXS + j]
    xb_host = np.ascontiguousarray(
        xm[:, :, PXA:].reshape(B, KO, P, NSB, PXS)
        .transpose(0, 3, 2, 1, 4)).reshape(B, NSB * P, KO * PXS)

    nc = _get_nc(MODE)
    in_maps = [
        {"w": w_host, "bias": bias_host, "xa": xa_host[i], "xb": xb_host[i]}
        for i in range(B)
    ]
    res = run_bass_kernel_spmd(nc, in_maps, core_ids=list(range(B)))

    out_full = np.empty((B, C, HW), dtype=np.float32)
    for i in range(B):
        oa = np.asarray(res.results[i]["outa"], dtype=np.float32)
        ob = np.asarray(res.results[i]["outb"], dtype=np.float32)
        out_full[i, :, :PXA] = oa                       # [(m p), j] == [c, j]
        # ob[s*P+p, m*PXS+j] -> y[m*128+p, PXA+s*128+j]
        ob = ob.reshape(NSB, P, MO, PXS).transpose(2, 1, 0, 3).reshape(C, HW - PXA)
        out_full[i, :, PXA:] = ob
    return np.ascontiguousarray(out_full.reshape(B, C, Hx, Wx))
